# revision 1
# baseline (speedup 1.0000x reference)
"""Trainium2 Bass kernel for GNN message passing (8-core SPMD, self-contained).

kernel(**inputs) -> np.ndarray [64, 1]

Strategy: edges sharded by destination-node range across the 8 NeuronCores
(edge/data parallel per the problem sharding hint, with the segment-sum
partials kept disjoint by dest-range so the per-step collective is a small
AllGather instead of a full AllReduce). Per step, each core computes
U = link_state @ Wm1a for its node slice in a transposed layout, AllGathers
the U table, gathers U rows per edge slot with indirect DMA (degree-sorted,
block-padded layout; one index per partition per call), applies
relu(u+v) = max(u,-v)+v so the per-edge bias-add folds into the segmented
reduction, reduces segments with a strided DVE reduce, folds Wm2/bm2 into a
node-level matmul, and updates the GRU on the node slice in place. The
readout gathers final states per graph and runs the 3-layer MLP transposed.
"""
import sys
sys.path.insert(0, '/opt/trn_rl_repo')
import numpy as np
import concourse.bass as bass
import concourse.bacc as bacc
import concourse.tile as tile
import concourse.mybir as mybir
from concourse.masks import make_identity
from concourse.bass_utils import run_bass_kernel_spmd
from bass_rust import add_dep_helper


def _dep(later, earlier, why):
    add_dep_helper(later.ins, earlier.ins, True, why)


NCORES = 8
N = 100000
M = 1600000
D = 32
H = 64
NUM_GRAPHS = 64
T = 4
BLK = 128          # nodes per block
GIL = 16           # U-row interleave group (tiles per DMA group)
KMAX = 64          # max gather idx columns per call
DUMMY_NEG = -1.0e30


def build_layout(states_first, states_second, states_graph_ids,
                 n=N, m=M, ncores=NCORES, num_graphs=NUM_GRAPHS):
    first = np.asarray(states_first, np.int64)
    second = np.asarray(states_second, np.int64)
    gids = np.asarray(states_graph_ids, np.int64)
    nc_real = n // ncores                      # 12500
    nc_pad = ((nc_real + BLK - 1) // BLK) * BLK  # 12544
    sl = nc_pad + 1                            # slice rows incl dummy
    npad_tot = sl * ncores                     # total table rows
    nblk = nc_pad // BLK                       # 98

    order = np.argsort(second, kind="stable")
    sfirst = first[order]
    ssecond = second[order]
    deg = np.bincount(second, minlength=n)

    # per-core pi: real nodes sorted by (deg, id), pads (deg 0) at end
    pi = np.empty((ncores, nc_pad), np.int64)   # pi[c, pos] = global node id or -1
    upos = np.full(n + 1, nc_pad, np.int64)     # node id -> table row (default dummy)
    blockC = np.zeros((ncores, nblk), np.int64)
    for c in range(ncores):
        ids = np.arange(c * nc_real, (c + 1) * nc_real)
        o = np.lexsort((ids, deg[ids]))
        ids_sorted = ids[o]
        row = np.concatenate([ids_sorted, -np.ones(nc_pad - nc_real, np.int64)])
        pi[c] = row
        d_sorted = np.concatenate([deg[ids_sorted], np.zeros(nc_pad - nc_real, np.int64)])
        blockC[c] = np.maximum.reduceat(d_sorted, np.arange(0, nc_pad, BLK))
    C_blocks = np.maximum(blockC.max(axis=0), 1)   # shared across cores

    # interleaved U row for pi position p: DMA group g holds gs=min(GIL,
    # nblk-GIL*g) tiles; node (tile t, partition q) lands at row
    # base(g) + q*gs + t  (so SBUF staging DMAs out fully contiguous).
    pos = np.arange(nc_pad)
    g = pos // (GIL * BLK)
    t = (pos // BLK) % GIL
    q = pos % BLK
    gs = np.minimum(GIL, nblk - GIL * g)
    base = BLK * GIL * g  # correct since all previous groups are full
    irow = base + q * gs + t
    for c in range(ncores):
        valid = pi[c] >= 0
        upos[pi[c, valid]] = c * sl + irow[valid]

    # groups of consecutive equal-C blocks, g*C <= KMAX
    groups = []   # (col0, blk0, nblks, C)
    col = 0
    b = 0
    while b < nblk:
        C = int(C_blocks[b])
        run = 1
        gmax = 1  # HW indirect DMA: one index per partition per call
        while b + run < nblk and C_blocks[b + run] == C and run < gmax:
            run += 1
        groups.append((col, b, run, C))
        col += run * C
        b += run
    totcols = col

    # gather idx per core: [BLK, totcols] int32 of U rows
    # slot (block bb, part p, c) -> edge = c-th edge of node pi[c][bb*BLK+p]
    edge_off = np.zeros(n + 1, np.int64)
    np.cumsum(deg, out=edge_off[1:])   # into ssecond-sorted arrays
    gidx = np.full((ncores, BLK, totcols), nc_pad, np.int32)
    for c in range(ncores):
        for (col0, blk0, gcnt, C) in groups:
            for j in range(gcnt):
                bb = blk0 + j
                nodes = pi[c, bb * BLK:(bb + 1) * BLK]
                for p in range(BLK):
                    node = nodes[p]
                    if node < 0:
                        continue
                    dg = deg[node]
                    e0 = edge_off[node]
                    src = sfirst[e0:e0 + dg]
                    gidx[c, p, col0 + j * C: col0 + j * C + dg] = upos[src]
    # degree column in pi order
    deg_pi = np.zeros((ncores, nc_pad), np.float32)
    for c in range(ncores):
        valid = pi[c] >= 0
        deg_pi[c, valid] = deg[pi[c, valid]]

    # readout: graph -> node list (pi/interleave positions in ls_tab)
    ng_core = num_graphs // ncores
    gsizes = np.bincount(gids, minlength=num_graphs)
    smax = int(max(1, ((gsizes.max() + BLK - 1) // BLK)))
    ridx = np.full((ncores, BLK, ng_core * smax), nc_pad, np.int32)
    gnodes_off = np.zeros(num_graphs + 1, np.int64)
    np.cumsum(gsizes, out=gnodes_off[1:])
    node_by_graph = np.argsort(gids, kind="stable")
    for c in range(ncores):
        for j in range(ng_core):
            gg = c * ng_core + j
            nodes = node_by_graph[gnodes_off[gg]:gnodes_off[gg + 1]]
            rows = upos[nodes]
            for s in range(len(rows)):
                ridx[c, s % BLK, j * smax + s // BLK] = rows[s]
    return dict(nc_real=nc_real, nc_pad=nc_pad, sl=sl, npad_tot=npad_tot, nblk=nblk,
                groups=groups, totcols=totcols, gidx=gidx, deg_pi=deg_pi,
                pi=pi, upos=upos, ng_core=ng_core, smax=smax, ridx=ridx,
                C_blocks=C_blocks)


def build_inputs_per_core(inp, lay, n=N, ncores=NCORES):
    """Returns list of per-core in_map dicts (numpy arrays)."""
    import ml_dtypes
    f32 = np.float32
    ls = np.asarray(inp["link_state"], f32)
    Wm1 = np.asarray(inp["Wm1"], f32); bm1 = np.asarray(inp["bm1"], f32)
    Wm2 = np.asarray(inp["Wm2"], f32); bm2 = np.asarray(inp["bm2"], f32)
    Wx = np.asarray(inp["Wx"], f32); Wh = np.asarray(inp["Wh"], f32)
    b_gru = np.asarray(inp["b_gru"], f32)
    d = ls.shape[1]; h = Wm1.shape[1]
    nc_pad = lay["nc_pad"]
    Wm1a, Wm1b = Wm1[:d], Wm1[d:]
    negWm1b_aug = np.vstack([-Wm1b, -bm1[None, :]]).astype(f32)      # [33, h]
    Wm2t = Wm2.astype(f32); bm2row = bm2[None, :].astype(f32)
    bz = (b_gru[0, :d] + b_gru[1, :d])[:, None].astype(f32)
    br = (b_gru[0, d:2*d] + b_gru[1, d:2*d])[:, None].astype(f32)
    b0h = b_gru[0, 2*d:][:, None].astype(f32)
    b1h = b_gru[1, 2*d:][:, None].astype(f32)
    rh = np.asarray(inp["Wr1"], f32).shape[1]
    br1m = np.stack([np.asarray(inp["br1"], f32).reshape(2, rh // 2)[i] for i in range(2)], 1)
    mask1m = np.asarray(inp["mask1"], f32).reshape(2, rh // 2).T.copy()
    br2m = np.asarray(inp["br2"], f32).reshape(2, rh // 2).T.copy()
    mask2m = np.asarray(inp["mask2"], f32).reshape(2, rh // 2).T.copy()
    # br1 reshaped wrong above; fix: halves are [0:128],[128:256]
    br1m = np.asarray(inp["br1"], f32).reshape(2, rh // 2).T.copy()

    maps = []
    for c in range(ncores):
        ls0T = np.zeros((d + 1, nc_pad), f32)
        valid = lay["pi"][c] >= 0
        # column order must match interleaved U rows? NO: lsT columns are pi
        # positions (block-major); U interleave happens at DMA staging time.
        ls0T[:d, valid] = ls[lay["pi"][c, valid]].T
        ls0T[d, :] = 1.0
        maps.append({
            "ls0T": ls0T,
            "gidx": lay["gidx"][c].astype(np.int32),
            "ridx": lay["ridx"][c].astype(np.int32),
            "deg_pi": lay["deg_pi"][c][None, :].astype(f32),
            "negWm1b_aug": negWm1b_aug,
            "Wm1a": Wm1a.astype(f32),
            "Wm2t": Wm2t, "bm2row": bm2row,
            "Wx": Wx, "Wh": Wh,
            "bz": bz, "br": br, "b0h": b0h, "b1h": b1h,
            "Wr1": np.asarray(inp["Wr1"], f32),
            "Wr2": np.asarray(inp["Wr2"], f32),
            "Wr3": np.asarray(inp["Wr3"], f32),
            "br1m": br1m, "mask1m": mask1m,
            "br2m": br2m, "mask2m": mask2m,
            "br3": np.asarray(inp["br3"], f32).reshape(1, 1),
        })
    return maps


F32 = mybir.dt.float32
BF16 = mybir.dt.bfloat16
I32 = mybir.dt.int32
AF = mybir.ActivationFunctionType
ALU = mybir.AluOpType
AX = mybir.AxisListType


F32 = mybir.dt.float32
BF16 = mybir.dt.bfloat16
I32 = mybir.dt.int32
AF = mybir.ActivationFunctionType
ALU = mybir.AluOpType
AX = mybir.AxisListType


def build_kernel(lay, d=32, h=64, rh=256, T=4, ncores=8, num_graphs=64,
                 gil=16, dummy_neg=-1.0e30, debug=False, edge_bf16=False):
    nc_pad = lay["nc_pad"]; nblk = lay["nblk"]; groups = lay["groups"]
    totcols = lay["totcols"]; ng_core = lay["ng_core"]; smax = lay["smax"]
    npad_tot = lay["npad_tot"]; sl = lay["sl"]
    cmax = max(g * C for (_, _, g, C) in groups)
    h3 = 3 * d  # GRU gate width (96)

    EDT = BF16 if edge_bf16 else F32
    nc = bacc.Bacc("TRN2", target_bir_lowering=False, debug=False,
                   num_devices=ncores)

    # ---- external inputs ----
    def ein(name, shape, dt=F32):
        return nc.dram_tensor(name, list(shape), dt, kind="ExternalInput")
    ls0T_e = ein("ls0T", [d + 1, nc_pad])
    gidx_e = ein("gidx", [128, totcols], I32)
    ridx_e = ein("ridx", [128, ng_core * smax], I32)
    deg_e = ein("deg_pi", [1, nc_pad])
    nWb_e = ein("negWm1b_aug", [d + 1, h])
    Wm1a_e = ein("Wm1a", [d, h])
    Wm2t_e = ein("Wm2t", [h, d])
    bm2r_e = ein("bm2row", [1, d])
    Wx_e = ein("Wx", [d, h3]); Wh_e = ein("Wh", [d, h3])
    bz_e = ein("bz", [d, 1]); br_e = ein("br", [d, 1])
    b0h_e = ein("b0h", [d, 1]); b1h_e = ein("b1h", [d, 1])
    Wr1_e = ein("Wr1", [d, rh]); Wr2_e = ein("Wr2", [rh, rh])
    Wr3_e = ein("Wr3", [rh, 1])
    br1_e = ein("br1m", [rh // 2, 2]); m1_e = ein("mask1m", [rh // 2, 2])
    br2_e = ein("br2m", [rh // 2, 2]); m2_e = ein("mask2m", [rh // 2, 2])
    br3_e = ein("br3", [1, 1])
    out_e = nc.dram_tensor("out_r", [1, num_graphs], F32, kind="ExternalOutput")
    dbg_e = [nc.dram_tensor(f"dbg_ls{t}", [d, nc_pad], F32, kind="ExternalOutput")
             for t in range(T)] if debug else None
    dbg_agg = nc.dram_tensor("dbg_aggT", [h + 1, nc_pad], F32,
                             kind="ExternalOutput") if debug else None

    with tile.TileContext(nc) as tc:
        with tc.tile_pool(name="const", bufs=1) as cp, \
             tc.tile_pool(name="dram", bufs=1, space="DRAM") as dp, \
             tc.tile_pool(name="ps", bufs=1, space="PSUM") as psp_, \
             tc.tile_pool(name="work", bufs=2) as wp, \
             tc.tile_pool(name="gruw", bufs=1) as gp:

            # ---- persistent DRAM ----
            U_slices = [dp.tile([sl, h], EDT, name=f"U_slice{i}")
                        for i in range(2)]
            U_tabs = [dp.tile([npad_tot, h], EDT, addr_space="Shared",
                              name=f"U_tab{t}") for t in range(T)]
            ls_slice = dp.tile([sl, d], F32)
            ls_tab = dp.tile([npad_tot, d], F32, addr_space="Shared")
            rg_slice = dp.tile([1, ng_core * d], F32)
            rg_all = dp.tile([ncores, ng_core * d], F32, addr_space="Shared")

            # ---- persistent SBUF ----
            lsA = cp.tile([d + 1, nc_pad], F32)
            vneg = cp.tile([128, nblk * h], EDT)
            ident = cp.tile([128, 128], F32)
            ones_c = cp.tile([128, 1], F32)
            nWb_s = cp.tile([d + 1, h], F32)
            Wm1a_s = cp.tile([d, h], F32)
            Wm2t_s = cp.tile([h, d], F32)
            bm2r_s = cp.tile([1, d], F32)
            Wx_s = cp.tile([d, h3], F32); Wh_s = cp.tile([d, h3], F32)
            bz_s = cp.tile([d, 1], F32); br_s = cp.tile([d, 1], F32)
            b0h_s = cp.tile([d, 1], F32); b1h_s = cp.tile([d, 1], F32)
            Wr1_s = cp.tile([d, rh], F32)
            Wr2a_s = cp.tile([rh // 2, rh], F32)
            Wr2b_s = cp.tile([rh // 2, rh], F32)
            Wr3_s = cp.tile([rh // 2, 2], F32)  # col j = Wr3[128j:128(j+1), 0]
            br1_s = cp.tile([rh // 2, 2], F32); m1_s = cp.tile([rh // 2, 2], F32)
            br2_s = cp.tile([rh // 2, 2], F32); m2_s = cp.tile([rh // 2, 2], F32)
            br3_s = cp.tile([1, 1], F32)
            dumU = cp.tile([1, h], EDT)
            dumL = cp.tile([1, d], F32)

            # ---- load constants ----
            make_identity(nc, ident[:])
            nc.vector.memset(ones_c[:], 1.0)
            nc.sync.dma_start(nWb_s[:], nWb_e.ap())
            nc.sync.dma_start(Wm1a_s[:], Wm1a_e.ap())
            nc.sync.dma_start(Wm2t_s[:], Wm2t_e.ap())
            nc.sync.dma_start(bm2r_s[:], bm2r_e.ap())
            nc.sync.dma_start(Wx_s[:], Wx_e.ap())
            nc.sync.dma_start(Wh_s[:], Wh_e.ap())
            nc.sync.dma_start(bz_s[:], bz_e.ap())
            nc.sync.dma_start(br_s[:], br_e.ap())
            nc.sync.dma_start(b0h_s[:], b0h_e.ap())
            nc.sync.dma_start(b1h_s[:], b1h_e.ap())
            nc.sync.dma_start(Wr1_s[:], Wr1_e.ap())
            nc.sync.dma_start(Wr2a_s[:], Wr2_e.ap()[0:rh // 2, :])
            nc.sync.dma_start(Wr2b_s[:], Wr2_e.ap()[rh // 2:, :])
            nc.sync.dma_start(Wr3_s[:], Wr3_e.ap().rearrange("(a b) c -> b (a c)", a=2))
            nc.sync.dma_start(br1_s[:], br1_e.ap())
            nc.sync.dma_start(m1_s[:], m1_e.ap())
            nc.sync.dma_start(br2_s[:], br2_e.ap())
            nc.sync.dma_start(m2_s[:], m2_e.ap())
            nc.sync.dma_start(br3_s[:], br3_e.ap())
            nc.vector.memset(dumU[:], dummy_neg)
            nc.sync.dma_start(U_slices[0][nc_pad:nc_pad + 1, :], dumU[:])
            nc.sync.dma_start(U_slices[1][nc_pad:nc_pad + 1, :], dumU[:])
            nc.vector.memset(dumL[:], 0.0)
            nc.sync.dma_start(ls_slice[nc_pad:nc_pad + 1, :], dumL[:])
            nc.sync.dma_start(lsA[:], ls0T_e.ap())

            cc_groups = [list(range(ncores))]
            ag_insts = []

            for step in range(T):
                U_slice = U_slices[step % 2]
                src = dst = lsA

                # ---- Vneg production ----
                for b8 in range(0, nblk, 8):
                    nb = min(8, nblk - b8)
                    ps = psp_.tile([128, 512], F32, tag="prod", bufs=2)
                    for j in range(nb):
                        b = b8 + j
                        nc.tensor.matmul(
                            ps[:, j * h:(j + 1) * h],
                            lhsT=src[0:d + 1, b * 128:(b + 1) * 128],
                            rhs=nWb_s[:], start=True, stop=True)
                    nc.scalar.activation(vneg[:, b8 * h:(b8 + nb) * h],
                                         ps[:, :nb * h], AF.Copy)

                # ---- U production + staged interleaved DMA ----
                for g0 in range(0, nblk, gil):
                    gsz = min(gil, nblk - g0)
                    stage = wp.tile([128, gil * h], EDT, tag="ustage")
                    for j8 in range(0, gsz, 8):
                        nb = min(8, gsz - j8)
                        ps = psp_.tile([128, 512], F32, tag="prod", bufs=2)
                        for j in range(nb):
                            b = g0 + j8 + j
                            nc.tensor.matmul(
                                ps[:, j * h:(j + 1) * h],
                                lhsT=src[0:d, b * 128:(b + 1) * 128],
                                rhs=Wm1a_s[:], start=True, stop=True)
                        nc.scalar.activation(stage[:, j8 * h:(j8 + nb) * h],
                                             ps[:, :nb * h], AF.Copy)
                    di = nc.sync.dma_start(
                        U_slice[g0 * 128:(g0 + gsz) * 128, :],
                        stage[:, :gsz * h])
                    if g0 == 0 and step >= 2:
                        _dep(di, ag_insts[step - 2], "U_slice WAR vs AG")

                # ---- AllGather U ----
                ag = nc.gpsimd.collective_compute(
                    "AllGather", ALU.bypass, replica_groups=cc_groups,
                    ins=[U_slice[:].opt()],
                    outs=[U_tabs[step][:].opt()])
                ag_insts.append(ag)
                first_gather = True

                # ---- edge stage ----
                for (col0, blk0, g, C) in groups:
                    k = g * C
                    gix = wp.tile([128, 64], I32, tag="gix")
                    nc.sync.dma_start(gix[:, :k], gidx_e.ap()[:, col0:col0 + k])
                    gt = wp.tile([128, cmax * h], EDT, tag="gt", bufs=3)
                    for c_ in range(k):
                        gi_ = nc.gpsimd.indirect_dma_start(
                            out=gt[:, c_ * h:(c_ + 1) * h], out_offset=None,
                            in_=U_tabs[step][:],
                            in_offset=bass.IndirectOffsetOnAxis(
                                ap=gix[:, c_:c_ + 1], axis=0))
                        if first_gather:
                            _dep(gi_, ag, "gather after U AllGather")
                            first_gather = False
                    gt4 = gt[:, :k * h].rearrange("p (j c f) -> p j c f", c=C, f=h)
                    vv = vneg[:, blk0 * h:(blk0 + g) * h] \
                        .rearrange("p (j f) -> p j f", f=h)[:, :, None, :] \
                        .to_broadcast([128, g, C, h])
                    nc.vector.tensor_tensor(out=gt4, in0=gt4, in1=vv, op=ALU.max)
                    sm = wp.tile([128, 8 * h], F32, tag="sm")
                    red_in = gt[:, :k * h].rearrange("p (j c f) -> p j f c", c=C, f=h)
                    nc.vector.tensor_reduce(out=sm[:, :g * h], in_=red_in,
                                            axis=AX.X, op=ALU.add)
                    cv = wp.tile([128, 8 * h], F32, tag="cv")
                    nc.vector.tensor_scalar_mul(cv[:, :g * h],
                                                vneg[:, blk0 * h:(blk0 + g) * h],
                                                -float(C))
                    agg = wp.tile([128, 8 * h], F32, tag="agg")
                    nc.vector.tensor_tensor(out=agg[:, :g * h], in0=sm[:, :g * h],
                                            in1=cv[:, :g * h], op=ALU.add)
                    # transpose blocks; flush ei+GRU every 4 blocks
                    for j in range(g):
                        b = blk0 + j
                        pos = b % 4
                        if pos == 0:
                            cur_pst = psp_.tile([128, 512], F32, tag="tr",
                                                bufs=2, name=f"pst{step}_{b}")
                        nc.tensor.transpose(
                            out=cur_pst[0:h, pos * 128:(pos + 1) * 128],
                            in_=agg[:, j * h:(j + 1) * h],
                            identity=ident[:])
                        if pos == 3 or b == nblk - 1:
                            used = (pos + 1) * 128
                            i0 = (b // 4) * 512
                            aggt = gp.tile([h, 512], F32, tag="aggt", bufs=3,
                                           name=f"aggt{step}_{b}")
                            nc.scalar.activation(aggt[:, :used],
                                                 cur_pst[0:h, :used], AF.Copy)
                            w = used
                            dg = gp.tile([1, 512], F32, tag="dg",
                                         name=f"dg{step}_{b}")
                            nc.sync.dma_start(dg[:, :w], deg_e.ap()[:, i0:i0 + w])
                            pse = psp_.tile([d, 512], F32, tag="ei",
                                            name=f"ei{step}_{b}")
                            nc.tensor.matmul(pse[:, :w], lhsT=Wm2t_s[:],
                                             rhs=aggt[0:h, :w],
                                             start=True, stop=False)
                            nc.tensor.matmul(pse[:, :w], lhsT=bm2r_s[:],
                                             rhs=dg[:, :w],
                                             start=False, stop=True)
                            eiT = gp.tile([d, 512], F32, tag="eiT")
                            nc.scalar.activation(eiT[:, :w], pse[:, :w], AF.Copy)
                            psZ = psp_.tile([96, 512], F32, tag="Z")
                            psS = psZ[0:2 * d, :]
                            nc.tensor.matmul(psS[:, :w], lhsT=Wx_s[:, 0:2 * d],
                                             rhs=eiT[:, :w], start=True, stop=False)
                            nc.tensor.matmul(psS[:, :w], lhsT=Wh_s[:, 0:2 * d],
                                             rhs=src[0:d, i0:i0 + w],
                                             start=False, stop=True)
                            psih = psZ[2 * d:96, :]
                            nc.tensor.matmul(psih[:, :w], lhsT=Wx_s[:, 2 * d:h3],
                                             rhs=eiT[:, :w], start=True, stop=True)
                            pshh = psp_.tile([d, 512], F32, tag="hh")
                            nc.tensor.matmul(pshh[:, :w], lhsT=Wh_s[:, 2 * d:h3],
                                             rhs=src[0:d, i0:i0 + w],
                                             start=True, stop=True)
                            z = gp.tile([d, 512], F32, tag="z")
                            nc.scalar.activation(z[:, :w], psZ[0:d, :w],
                                                 AF.Sigmoid, bias=bz_s[:])
                            r = gp.tile([d, 512], F32, tag="r")
                            nc.scalar.activation(r[:, :w], psZ[d:2 * d, :w],
                                                 AF.Sigmoid, bias=br_s[:])
                            t1 = gp.tile([d, 512], F32, tag="t1")
                            nc.vector.tensor_scalar_add(t1[:, :w], pshh[:, :w],
                                                        b1h_s[:])
                            t2 = gp.tile([d, 512], F32, tag="t2")
                            nc.vector.tensor_tensor(out=t2[:, :w], in0=r[:, :w],
                                                    in1=t1[:, :w], op=ALU.mult)
                            t3 = gp.tile([d, 512], F32, tag="t3")
                            nc.vector.tensor_tensor(out=t3[:, :w], in0=t2[:, :w],
                                                    in1=psZ[2 * d:96, :w],
                                                    op=ALU.add)
                            hh = gp.tile([d, 512], F32, tag="hhat")
                            nc.scalar.activation(hh[:, :w], t3[:, :w], AF.Tanh,
                                                 bias=b0h_s[:])
                            d_ = gp.tile([d, 512], F32, tag="d_")
                            nc.vector.tensor_tensor(out=d_[:, :w],
                                                    in0=src[0:d, i0:i0 + w],
                                                    in1=hh[:, :w], op=ALU.subtract)
                            e_ = gp.tile([d, 512], F32, tag="e_")
                            nc.vector.tensor_tensor(out=e_[:, :w], in0=z[:, :w],
                                                    in1=d_[:, :w], op=ALU.mult)
                            nc.vector.tensor_tensor(out=dst[0:d, i0:i0 + w],
                                                    in0=hh[:, :w],
                                                    in1=e_[:, :w], op=ALU.add)

                if debug:
                    for i0 in range(0, nc_pad, 512):
                        w = min(512, nc_pad - i0)
                        nc.sync.dma_start(dbg_e[step].ap()[:, i0:i0 + w],
                                          dst[0:d, i0:i0 + w])

            # ---- final: ls row-major + AllGather ----
            fin = lsA
            for g0 in range(0, nblk, gil):
                gsz = min(gil, nblk - g0)
                stage = wp.tile([128, gil * d], F32, tag="lstage")
                j = 0
                while j < gsz:
                    nt = min(16, gsz - j)
                    pst = psp_.tile([128, 512], F32, tag="tr", bufs=2)
                    for jj in range(nt):
                        b = g0 + j + jj
                        nc.tensor.transpose(
                            out=pst[:, jj * d:(jj + 1) * d],
                            in_=fin[0:d, b * 128:(b + 1) * 128],
                            identity=ident[0:d, 0:d])
                    nc.scalar.activation(stage[:, j * d:(j + nt) * d],
                                         pst[:, :nt * d], AF.Copy)
                    j += nt
                nc.sync.dma_start(ls_slice[g0 * 128:(g0 + gsz) * 128, :],
                                  stage[:, :gsz * d])
            ag_ls = nc.gpsimd.collective_compute(
                "AllGather", ALU.bypass, replica_groups=cc_groups,
                ins=[ls_slice[:].opt()],
                outs=[ls_tab[:].opt()])

            # ---- readout ----
            ridx_s = cp.tile([128, ng_core * smax], I32)
            nc.sync.dma_start(ridx_s[:], ridx_e.ap())
            parts = cp.tile([128, ng_core * d], F32)
            for j in range(ng_core):
                rt = wp.tile([128, smax * d], F32, tag="rt")
                for s_ in range(smax):
                    gr_ = nc.gpsimd.indirect_dma_start(
                        out=rt[:, s_ * d:(s_ + 1) * d], out_offset=None,
                        in_=ls_tab[:],
                        in_offset=bass.IndirectOffsetOnAxis(
                            ap=ridx_s[:, j * smax + s_:j * smax + s_ + 1],
                            axis=0))
                    if j == 0 and s_ == 0:
                        _dep(gr_, ag_ls, "readout gather after ls AllGather")
                rin = rt[:].rearrange("p (s f) -> p f s", f=d)
                nc.vector.tensor_reduce(out=parts[:, j * d:(j + 1) * d],
                                        in_=rin, axis=AX.X, op=ALU.add)
            psp = psp_.tile([128, 512], F32, tag="ro", name="ro_pp")[0:1, 0:ng_core * d]
            nc.tensor.matmul(psp[:], lhsT=ones_c[:], rhs=parts[:],
                             start=True, stop=True)
            prs = cp.tile([1, ng_core * d], F32)
            nc.vector.tensor_copy(prs[:], psp[:])
            nc.sync.dma_start(rg_slice[:, :], prs[:])
            ag_rg = nc.gpsimd.collective_compute(
                "AllGather", ALU.bypass, replica_groups=cc_groups,
                ins=[rg_slice[0:1, :].opt()],
                outs=[rg_all[0:ncores, :].opt()])
            P_sb = cp.tile([num_graphs, d], F32)
            dP = nc.sync.dma_start(
                P_sb[:], rg_all[:].rearrange("a (b c) -> (a b) c", c=d))
            _dep(dP, ag_rg, "P load after rg AllGather")
            psq = psp_.tile([128, 512], F32, tag="ro", name="ro_pt")[0:d, 0:num_graphs]
            nc.tensor.transpose(out=psq[:], in_=P_sb[:], identity=ident[0:num_graphs, 0:num_graphs])
            PT = cp.tile([d, num_graphs], F32)
            nc.vector.tensor_copy(PT[:], psq[:])
            # r1 = relu(P@Wr1 + br1)*mask1  (transposed, halves on partitions)
            r1m = []
            for hf in range(2):
                ps1 = psp_.tile([128, 512], F32, tag="ro", name="ro_r1")[0:rh // 2, 0:num_graphs]
                nc.tensor.matmul(ps1[:], lhsT=Wr1_s[:, hf * 128:(hf + 1) * 128],
                                 rhs=PT[:], start=True, stop=True)
                r1h = cp.tile([rh // 2, num_graphs], F32, name=f"r1h{hf}")
                nc.scalar.activation(r1h[:], ps1[:], AF.Relu,
                                     bias=br1_s[:, hf:hf + 1])
                r1x = cp.tile([rh // 2, num_graphs], F32, name=f"r1x{hf}")
                nc.vector.tensor_scalar_mul(r1x[:], r1h[:], m1_s[:, hf:hf + 1])
                r1m.append(r1x)
            r2m = []
            for hf in range(2):
                ps2 = psp_.tile([128, 512], F32, tag="ro", name="ro_r2")[0:rh // 2, 0:num_graphs]
                nc.tensor.matmul(ps2[:], lhsT=Wr2a_s[:, hf * 128:(hf + 1) * 128],
                                 rhs=r1m[0][:], start=True, stop=False)
                nc.tensor.matmul(ps2[:], lhsT=Wr2b_s[:, hf * 128:(hf + 1) * 128],
                                 rhs=r1m[1][:], start=False, stop=True)
                r2h = cp.tile([rh // 2, num_graphs], F32, name=f"r2h{hf}")
                nc.scalar.activation(r2h[:], ps2[:], AF.Relu,
                                     bias=br2_s[:, hf:hf + 1])
                r2x = cp.tile([rh // 2, num_graphs], F32, name=f"r2x{hf}")
                nc.vector.tensor_scalar_mul(r2x[:], r2h[:], m2_s[:, hf:hf + 1])
                r2m.append(r2x)
            ps3 = psp_.tile([128, 512], F32, tag="ro", name="ro_r3")[0:1, 0:num_graphs]
            nc.tensor.matmul(ps3[:], lhsT=Wr3_s[:, 0:1], rhs=r2m[0][:],
                             start=True, stop=False)
            nc.tensor.matmul(ps3[:], lhsT=Wr3_s[:, 1:2], rhs=r2m[1][:],
                             start=False, stop=True)
            outs = cp.tile([1, num_graphs], F32)
            nc.vector.tensor_scalar_add(outs[:], ps3[:], br3_s[:])
            nc.sync.dma_start(out_e.ap(), outs[:])

    nc.compile()
    return nc


def kernel(**inputs):
    inp = {k: (np.asarray(v) if not np.isscalar(v) else v)
           for k, v in inputs.items()}
    lay = build_layout(inp["states_first"], inp["states_second"],
                       inp["states_graph_ids"])
    maps = build_inputs_per_core(inp, lay)
    nc = build_kernel(lay)
    res = run_bass_kernel_spmd(nc, maps, core_ids=list(range(NCORES)))
    out = np.asarray(res.results[0]["out_r"], np.float32).reshape(-1, 1)
    return out

